# revision 56
# baseline (speedup 1.0000x reference)
"""Trainium2 Bass kernel for conv-qkv rank-1 attention.

out = gamma * (q+bq) * sum((k+bk)*(v+bv)) + x, where q,k,v are
per-time-slice 3x3 convs (C=64 -> C=64) of x [B=8, C=64, T=16, W=64, H=64].

Sharding: data-parallel over B across 8 cores (1 example/core), conv
weights replicated. No cross-core communication.

Design (~285us vs ~492us v1 baseline; PE-bound at the hardware floor):
- Slice pair per pass: slice t on SBUF partitions 0-63, t+1 on 64-127;
  the two 64-row PE tile chains stream concurrently (input port limit:
  two K=64 streams saturate the 128-partition rhs port).
- 18 pair-slots per (pair, 512-px block): 9 taps x ([k|q] M=128 +
  [v|v] M=128). No bias taps: bq/bv fold into the ACT evacuation bias,
  bk into the DVE STT op0-add scalar, gamma into wv/bv host-side.
- All matmuls are 64x128 tiles (v weights column-duplicated): mixing
  64x64 and 64x128 shapes flips the PE tiling mode, whose drain broke
  fill/drain overlap (222 -> 218 ns/slot once uniform; theoretical
  floor is 512/2.4GHz + ~3 NX cycles = 216).
- bf16 end-to-end (PE is 1 cyc/row for both f32r and bf16, but bf16
  LDWEIGHTS is half the load and draws less power -> less HAM
  throttling; absmax rel err 5.3e-3 vs 2e-2 gate).
- Hi chain stationary flipped to [Wq|Wk] so the k*v STT, s accumulator
  and q all live on partitions 64-127: no cross-partition s swap.
- Host pads H to 66 so each x slice loads with ONE contiguous
  descriptor per partition (was 64 x 256B strided descriptors -> 57us
  serial startup and 205us of DMA activity).
- Startup: matmuls wait on per-queue DMA completion counters (+~4.5us
  sem latency), so wv goes first on sync, the first pair is split
  across all 3 DMA queues, and later loads are emitted behind the
  first blocks' matmuls. First matmul at ~13us.
- Epilogue per pair: one [128,8] reduce -> s, merged [128,512]
  q*s+x STTs on DVE, out-DMAs on sync.
"""

import os

import numpy as np

import concourse.bacc as bacc
import concourse.bass as bass
import concourse.mybir as mybir
import concourse.tile as tile
from concourse import bass_utils

F32 = mybir.dt.float32
F32R = mybir.dt.float32r
BF16 = mybir.dt.bfloat16
ALU = mybir.AluOpType
ACTF = mybir.ActivationFunctionType

B, C, T, W, H = 8, 64, 16, 64, 64
HP = H + 2                     # host-padded H
WP = W + 2                     # SBUF-padded W rows
NPAIR = T // 2                 # slice pairs per core
RB = 8                         # W-rows per pixel block
NBLK = W // RB                 # pixel blocks per slice
BN = RB * H                    # moving free dim per matmul (512)
NTAP = 9                       # conv taps (no bias tap)

XDT_NAME = os.environ.get("BASS_XDT", "bf16")   # moving/x dtype
# walrus rejects mixed 32/16-bit matmul inputs: stationary follows moving
WDT_NAME = os.environ.get("BASS_WDT", "bf16" if XDT_NAME == "bf16" else "f32r")
# out/qs storage dtype follows x by default
ODT_NAME = os.environ.get("BASS_ODT", XDT_NAME)
# GPSIMD cannot access PSUM (BIR verifier) -> evacuations must use ACT
VEVAC = os.environ.get("BASS_VEVAC", "act")     # pool | act
# v matmul layout: "dup" = M=128 [Wv|Wv] so every matmul is 64x128 and
# the PE never switches tiling mode (mode flips cost an array drain);
# "quad" = M=64 with v-hi at tile col 64 sharing one bank
VMODE = os.environ.get("BASS_VMODE", "dup")     # dup | quad | split
# Pool rejects TensorScalarPtr at codegen -> out-STT stays on DVE
POOLOUT = os.environ.get("BASS_POOLOUT", "0") == "1"
# carry the 66-wide H padding through qs/ot/out so every out-STT operand
# is stride-1. Measured: no gain (TensorScalarPtr has no 2x uop; out-STT
# stays ~741ns either way), so default off for the simpler layout.
OUT66 = os.environ.get("BASS_OUT66", "0") == "1"
OW = 66 if OUT66 else H


def _round22(a: np.ndarray) -> np.ndarray:
    """Round fp32 to 11 mantissa bits so the PE's FP22 read-truncation is
    exact (unbiased quantization instead of truncation)."""
    u = np.ascontiguousarray(a, np.float32).view(np.uint32).astype(np.uint64)
    u = ((u + 0x800) & 0xFFFFF000).astype(np.uint32)
    return u.view(np.float32)


def _to_bf16(a: np.ndarray) -> np.ndarray:
    import ml_dtypes
    return np.ascontiguousarray(a, np.float32).astype(ml_dtypes.bfloat16)


def _pack_w(a: np.ndarray) -> np.ndarray:
    return _to_bf16(a) if WDT_NAME == "bf16" else _round22(a)


def _pack_weights(wq, wk, wv, bq, bk, bv, gamma):
    """Pack stationary operands (no bias rows; gamma folded into wv/bv).

    wkq [128, 9, 128]: [Wk | Wq] on both partition halves (k lands on
    psum partitions 0-63 for the DVE accum op, q on 64-127).
    wv2 [128, 9, 64]: gamma*Wv on both halves (M=64).
    bias [128, 3]: col0=bq, col1=bk, col2=gamma*bv, duplicated halves.
    """
    g = float(np.asarray(gamma).reshape(-1)[0])

    def taps(w):  # [O, I, 1, 3, 3] -> [I, 9, O]
        return np.ascontiguousarray(
            w.reshape(C, C, 9).transpose(1, 2, 0), np.float32)

    wq_t, wk_t, wv_t = taps(wq), taps(wk), taps(wv) * g
    # lo chain: [Wk | Wq] (k on psum partitions 0-63); hi chain flipped
    # to [Wq | Wk] so k_{t+1} lands on partitions 64-127 and the whole
    # hi k*v/s pipeline stays on the upper partition half (no s swap)
    wkq = np.zeros((128, NTAP, 128), np.float32)
    wkq[0:64, :, 0:64] = wk_t
    wkq[0:64, :, 64:128] = wq_t
    wkq[64:128, :, 0:64] = wq_t
    wkq[64:128, :, 64:128] = wk_t

    if VMODE == "dup":
        wv2 = np.zeros((128, NTAP, 128), np.float32)
        wv2[0:64, :, 0:64] = wv_t
        wv2[0:64, :, 64:128] = wv_t
        wv2[64:128, :, 0:64] = wv_t
        wv2[64:128, :, 64:128] = wv_t
    else:
        wv2 = np.zeros((128, NTAP, 64), np.float32)
        wv2[0:64] = wv_t
        wv2[64:128] = wv_t

    bias = np.zeros((128, 3), np.float32)
    bias[0:64, 0] = bq
    bias[64:128, 0] = bq
    bias[0:64, 1] = bk
    bias[64:128, 1] = bk
    bias[0:64, 2] = bv * g
    bias[64:128, 2] = bv * g
    return _pack_w(wkq), _pack_w(wv2), bias


def _emit(nc, tc, x_d, wkq_d, wv_d, bias_d, zer_d, out_d, ctx):
    xdt = F32R if XDT_NAME == "f32r" else BF16  # storage dtype of x tiles

    const = ctx.enter_context(tc.tile_pool(name="const", bufs=1))
    state = ctx.enter_context(tc.tile_pool(name="state", bufs=1))
    # 8 PSUM banks total: quad -> kq triple-buffered (6) + v shared (2);
    # dup/split -> kq double (4) + v double (4)
    psum = ctx.enter_context(
        tc.tile_pool(name="psum", bufs=3 if VMODE == "quad" else 2,
                     space=bass.MemorySpace.PSUM))
    psumv = ctx.enter_context(
        tc.tile_pool(name="psumv", bufs=2, space=bass.MemorySpace.PSUM))
    vpool = ctx.enter_context(tc.tile_pool(name="vpool", bufs=2))

    wdt = BF16 if WDT_NAME == "bf16" else F32R
    wkq_t = const.tile([128, NTAP, 128], wdt, tag="wkq")
    wv_t = const.tile([128, NTAP, 128 if VMODE == "dup" else 64], wdt,
                      tag="wv")
    bias_t = const.tile([128, 3], F32, tag="bias")

    odt = F32 if ODT_NAME == "f32r" else BF16

    xp = [state.tile([128, WP, HP], xdt, tag=f"xp{i}", name=f"xp{i}")
          for i in range(3)]
    qs = [state.tile([128, W, OW], odt, tag=f"qs{i}", name=f"qs{i}")
          for i in range(2)]
    ot = [state.tile([128, W, OW], odt, tag=f"ot{i}", name=f"ot{i}")
          for i in range(2)]
    scr = state.tile([128, BN], F32, tag="scr", name="scr")
    sparts = [state.tile([128, NBLK], F32, tag=f"sp{i}", name=f"sp{i}")
              for i in range(2)]
    sfull = [state.tile([128, 1], F32, tag=f"sf{i}", name=f"sf{i}")
             for i in range(2)]

    def load_pair(p):
        t_ = xp[p % 3]
        nc.sync.dma_start(t_[0:64, 1:1 + W, :], x_d[:, 2 * p])
        nc.sync.dma_start(t_[64:128, 1:1 + W, :], x_d[:, 2 * p + 1])

    # wv first on sync (its completion sem gates the first matmul; DMA
    # completion sems lag the transfer by ~4us, so head-of-queue matters)
    nc.sync.dma_start(wv_t[:], wv_d[:])
    # HAM warm-up: burn the free-running 3.4us half-speed window on dummy
    # matmuls while the first loads are in flight. The dummies MUST be the
    # same 64x128 tile shape as every real matmul: a 64x64-shaped attempt
    # flipped the PE tiling mode and regressed the whole body 222->266
    # ns/slot.
    if xdt == BF16 and os.environ.get("BASS_WARM", "1") == "1":
        warm = state.tile([128, BN], xdt, tag="warm", name="warm")
        nc.vector.memset(warm[:, :], 0.0)
        wps = psum.tile([128, BN], F32, tag="kq_lo")
        for _ in range(16):
            nc.tensor.matmul(wps[:, :], warm[0:64, 0:128], warm[0:64, :],
                             start=True, stop=True)
    # zero the W-pad rows once (H-pad columns come zeroed from the host).
    # The BIR verifier rejects compute-engine writes feeding an fp32r
    # matmul, so in f32r mode the zeros come from a host tensor via DMA
    # (on the ACT queue, off the x-load path).
    for t_ in xp:
        if xdt == F32R:
            nc.scalar.dma_start(t_[:, 0, :], zer_d[:, :])
            nc.scalar.dma_start(t_[:, WP - 1, :], zer_d[:, :])
        else:
            nc.vector.memset(t_[:, 0, :], 0.0)
            nc.vector.memset(t_[:, WP - 1, :], 0.0)
    if OUT66:
        # the out-STT reads qs pad columns that the evacs never write
        for qt in qs:
            nc.vector.memset(qt[:, :, 0], 0.0)
            nc.vector.memset(qt[:, :, OW - 1], 0.0)
    # first pair split across all three DMA-capable queues so the first
    # matmul can start after a ~2us quarter-slice load
    HW2 = W // 2
    nc.gpsimd.dma_start(xp[0][0:64, 1:1 + HW2, :], x_d[:, 0, 0:HW2])
    nc.scalar.dma_start(xp[0][0:64, 1 + HW2:1 + W, :], x_d[:, 0, HW2:W])
    nc.sync.dma_start(xp[0][64:128, 1:1 + HW2, :], x_d[:, 1, 0:HW2])
    nc.sync.dma_start(xp[0][64:128, 1 + HW2:1 + W, :], x_d[:, 1, HW2:W])
    nc.gpsimd.dma_start(wkq_t[:], wkq_d[:])
    nc.gpsimd.dma_start(bias_t[:], bias_d[:])
    # load_pair(1) is emitted inside pair 0's block loop: matmuls wait on
    # the issuing queue's DMA counter, so any DMA emitted earlier on the
    # same queue delays the first matmul

    def mm_rhs(xp_, half, tap, j):
        dy, dx = tap // 3, tap % 3
        r0 = j * RB + dy
        return xp_[64 * half:64 * half + 64, r0:r0 + RB, dx:dx + H]

    for p in range(NPAIR):
        pb = p % 2
        xp_, qs_, ot_ = xp[p % 3], qs[pb], ot[pb]

        if p + 2 < NPAIR:
            load_pair(p + 2)

        for j in range(NBLK):
            if p == 0 and j == 2:
                load_pair(1)
            if p == 0 and j == 5:
                load_pair(2)
            if VMODE == "quad":
                v_lo = v_hi = psumv.tile([128, BN], F32, tag="v_lo",
                                         name="v_lo")
                v_lo_out, v_hi_out = v_lo[0:64, :], v_hi[64:128, :]
            elif VMODE == "dup":
                v_lo = psumv.tile([128, BN], F32, tag="v_lo", name="v_lo")
                v_hi = psumv.tile([128, BN], F32, tag="v_hi", name="v_hi")
                v_lo_out, v_hi_out = v_lo[:, :], v_hi[:, :]
            else:
                v_lo = psumv.tile([128, BN], F32, tag="v_lo", name="v_lo")
                v_hi = psumv.tile([128, BN], F32, tag="v_hi", name="v_hi")
                v_lo_out, v_hi_out = v_lo[0:64, :], v_hi[0:64, :]
            kq_lo = psum.tile([128, BN], F32, tag="kq_lo")
            kq_hi = psum.tile([128, BN], F32, tag="kq_hi")

            for tap in range(NTAP):
                st, sp = tap == 0, tap == NTAP - 1
                nc.tensor.matmul(
                    v_lo_out, wv_t[0:64, tap, :],
                    mm_rhs(xp_, 0, tap, j), start=st, stop=sp)
                nc.tensor.matmul(
                    v_hi_out, wv_t[64:128, tap, :],
                    mm_rhs(xp_, 1, tap, j), start=st, stop=sp)
            for tap in range(NTAP):
                st, sp = tap == 0, tap == NTAP - 1
                nc.tensor.matmul(
                    kq_lo[:, :], wkq_t[0:64, tap, :],
                    mm_rhs(xp_, 0, tap, j), start=st, stop=sp)
                nc.tensor.matmul(
                    kq_hi[:, :], wkq_t[64:128, tap, :],
                    mm_rhs(xp_, 1, tap, j), start=st, stop=sp)

            # v + bv -> SBUF (ACT), q + bq -> SBUF (ACT; lo crosses
            # partitions 64-127 -> 0-63 to line up with x_t)
            vsb = vpool.tile([128, BN], F32, tag="vsb", name="vsb")
            if VMODE == "quad":
                nc.scalar.activation(
                    vsb[:, :], v_lo[:, :], ACTF.Identity,
                    bias=bias_t[:, 2:3])
            elif VMODE == "dup":
                # dup layout puts v_{t+1} on partitions 64-127 of its own
                # bank too, so both evacs are partition-aligned
                nc.scalar.activation(
                    vsb[0:64, :], v_lo[0:64, :], ACTF.Identity,
                    bias=bias_t[0:64, 2:3])
                nc.scalar.activation(
                    vsb[64:128, :], v_hi[64:128, :], ACTF.Identity,
                    bias=bias_t[64:128, 2:3])
            else:
                nc.scalar.activation(
                    vsb[0:64, :], v_lo[0:64, :], ACTF.Identity,
                    bias=bias_t[0:64, 2:3])
                nc.scalar.activation(
                    vsb[64:128, :], v_hi[0:64, :], ACTF.Identity,
                    bias=bias_t[64:128, 2:3])
            qoff = 1 if OUT66 else 0
            nc.scalar.activation(
                qs_[0:64, j * RB:(j + 1) * RB, qoff:qoff + H],
                kq_lo[64:128, :], ACTF.Identity, bias=bias_t[64:128, 0:1])
            nc.scalar.activation(
                qs_[64:128, j * RB:(j + 1) * RB, qoff:qoff + H],
                kq_hi[0:64, :], ACTF.Identity, bias=bias_t[0:64, 0:1])

            # (k+bk)*v with pixel-sum accumulation; the lo chain lives on
            # partitions 0-63, the hi chain on 64-127 throughout
            nc.vector.scalar_tensor_tensor(
                out=scr[0:64, :], in0=kq_lo[0:64, :],
                scalar=bias_t[0:64, 1:2], in1=vsb[0:64, :],
                op0=ALU.add, op1=ALU.mult,
                accum_out=sparts[pb][0:64, j:j + 1])
            nc.vector.scalar_tensor_tensor(
                out=scr[64:128, :], in0=kq_hi[64:128, :],
                scalar=bias_t[64:128, 1:2], in1=vsb[64:128, :],
                op0=ALU.add, op1=ALU.mult,
                accum_out=sparts[pb][64:128, j:j + 1])

        nc.vector.tensor_reduce(
            sfull[pb][:, 0:1], sparts[pb][:, :],
            axis=mybir.AxisListType.X, op=ALU.add)

        for j in range(NBLK):
            if OUT66:
                # every operand stride-1 over the 66-wide padded rows
                in1 = xp_[:, 1 + j * RB:1 + (j + 1) * RB, :]
                out_ap = ot_[:, j * RB:(j + 1) * RB, :]
                in0_ap = qs_[:, j * RB:(j + 1) * RB, :]
            else:
                in1 = xp_[:, 1 + j * RB:1 + (j + 1) * RB, 1:1 + H]
                out_ap = ot_[:, j * RB:(j + 1) * RB, 0:H]
                in0_ap = qs_[:, j * RB:(j + 1) * RB, 0:H]
            if xdt == F32R:
                in1 = in1.bitcast(F32)
            nc.vector.scalar_tensor_tensor(
                out=out_ap,
                in0=in0_ap,
                scalar=sfull[pb][:, 0:1],
                in1=in1,
                op0=ALU.mult, op1=ALU.add)

        # out-DMAs split across queues: the final barrier waits each
        # queue's last completion sem (~4.5us); two DMAs on one queue
        # serialize those waits at the kernel tail
        nc.sync.dma_start(out_d[:, 2 * p], ot_[0:64, :])
        nc.scalar.dma_start(out_d[:, 2 * p + 1], ot_[64:128, :])


_CACHE = {}


def _build():
    key = (XDT_NAME, WDT_NAME, VEVAC, VMODE)
    if key in _CACHE:
        return _CACHE[key]
    nc = bacc.Bacc("TRN2", target_bir_lowering=False, debug=False,
                   enable_asserts=False, num_devices=8)
    xdt = F32R if XDT_NAME == "f32r" else BF16
    wdt = BF16 if WDT_NAME == "bf16" else F32R
    x_d = nc.dram_tensor("xpad", (C, T, W, HP), xdt,
                         kind="ExternalInput").ap()
    wkq_d = nc.dram_tensor("wkq", (128, NTAP, 128), wdt,
                           kind="ExternalInput").ap()
    wv_d = nc.dram_tensor("wv2", (128, NTAP, 128 if VMODE == "dup" else 64),
                          wdt, kind="ExternalInput").ap()
    bias_d = nc.dram_tensor("biases", (128, 3), F32,
                            kind="ExternalInput").ap()
    zer_d = nc.dram_tensor("zer", (128, HP), xdt,
                           kind="ExternalInput").ap()
    odt = F32 if ODT_NAME == "f32r" else BF16
    out_d = nc.dram_tensor("out", (C, T, W, OW), odt,
                           kind="ExternalOutput").ap()
    from contextlib import ExitStack
    with tile.TileContext(nc) as tc, ExitStack() as ctx:
        _emit(nc, tc, x_d, wkq_d, wv_d, bias_d, zer_d, out_d, ctx)
    nc.compile()
    _CACHE[key] = nc
    return nc


def run_spmd(x, wq, wk, wv, bq, bk, bv, gamma, trace=False, **kw):
    nc = _build()
    wkq, wv2, biases = _pack_weights(
        np.asarray(wq, np.float32), np.asarray(wk, np.float32),
        np.asarray(wv, np.float32), np.asarray(bq, np.float32),
        np.asarray(bk, np.float32), np.asarray(bv, np.float32),
        np.asarray(gamma, np.float32))
    x = np.asarray(x, np.float32)
    xpad = np.zeros((B, C, T, W, HP), np.float32)
    xpad[..., 1:1 + H] = x
    zer = np.zeros((128, HP), np.float32)
    if XDT_NAME == "bf16":
        xpad = _to_bf16(xpad)
        zer = _to_bf16(zer)
    in_maps = [
        {"xpad": np.ascontiguousarray(xpad[b]), "wkq": wkq, "wv2": wv2,
         "biases": biases, "zer": zer}
        for b in range(B)
    ]
    res = bass_utils.run_bass_kernel_spmd(
        nc, in_maps, core_ids=list(range(B)), trace=trace, **kw)
    out = np.stack(
        [np.asarray(res.results[b]["out"], np.float32) for b in range(B)],
        axis=0)
    if OUT66:
        out = np.ascontiguousarray(out[..., 1:1 + H])
    return out, res


def kernel(x, wq, wk, wv, bq, bk, bv, gamma):
    out, _ = run_spmd(x, wq, wk, wv, bq, bk, bv, gamma)
    return out


# revision 57
# speedup vs baseline: 1.0215x; 1.0215x over previous
"""Trainium2 Bass kernel for conv-qkv rank-1 attention.

out = gamma * (q+bq) * sum((k+bk)*(v+bv)) + x, where q,k,v are
per-time-slice 3x3 convs (C=64 -> C=64) of x [B=8, C=64, T=16, W=64, H=64].

Sharding: data-parallel over B across 8 cores (1 example/core), conv
weights replicated. No cross-core communication.

Design (~285us vs ~492us v1 baseline; PE-bound at the hardware floor):
- Slice pair per pass: slice t on SBUF partitions 0-63, t+1 on 64-127;
  the two 64-row PE tile chains stream concurrently (input port limit:
  two K=64 streams saturate the 128-partition rhs port).
- 18 pair-slots per (pair, 512-px block): 9 taps x ([k|q] M=128 +
  [v|v] M=128). No bias taps: bq/bv fold into the ACT evacuation bias,
  bk into the DVE STT op0-add scalar, gamma into wv/bv host-side.
- All matmuls are 64x128 tiles (v weights column-duplicated): mixing
  64x64 and 64x128 shapes flips the PE tiling mode, whose drain broke
  fill/drain overlap (222 -> 218 ns/slot once uniform; theoretical
  floor is 512/2.4GHz + ~3 NX cycles = 216).
- bf16 end-to-end (PE is 1 cyc/row for both f32r and bf16, but bf16
  LDWEIGHTS is half the load and draws less power -> less HAM
  throttling; absmax rel err 5.3e-3 vs 2e-2 gate).
- Hi chain stationary flipped to [Wq|Wk] so the k*v STT, s accumulator
  and q all live on partitions 64-127: no cross-partition s swap.
- Host pads H to 66 so each x slice loads with ONE contiguous
  descriptor per partition (was 64 x 256B strided descriptors -> 57us
  serial startup and 205us of DMA activity).
- Startup: matmuls wait on per-queue DMA completion counters (+~4.5us
  sem latency), so wv goes first on sync, the first pair is split
  across all 3 DMA queues, and later loads are emitted behind the
  first blocks' matmuls. First matmul at ~13us.
- Epilogue per pair: one [128,8] reduce -> s, merged [128,512]
  q*s+x STTs on DVE, out-DMAs on sync.
"""

import os

import numpy as np

import concourse.bacc as bacc
import concourse.bass as bass
import concourse.mybir as mybir
import concourse.tile as tile
from concourse import bass_utils

F32 = mybir.dt.float32
F32R = mybir.dt.float32r
BF16 = mybir.dt.bfloat16
ALU = mybir.AluOpType
ACTF = mybir.ActivationFunctionType

B, C, T, W, H = 8, 64, 16, 64, 64
HP = H + 2                     # host-padded H
WP = W + 2                     # SBUF-padded W rows
NPAIR = T // 2                 # slice pairs per core
RB = 8                         # W-rows per pixel block
NBLK = W // RB                 # pixel blocks per slice
BN = RB * H                    # moving free dim per matmul (512)
NTAP = 9                       # conv taps (no bias tap)

XDT_NAME = os.environ.get("BASS_XDT", "bf16")   # moving/x dtype
# walrus rejects mixed 32/16-bit matmul inputs: stationary follows moving
WDT_NAME = os.environ.get("BASS_WDT", "bf16" if XDT_NAME == "bf16" else "f32r")
# out/qs storage dtype follows x by default
ODT_NAME = os.environ.get("BASS_ODT", XDT_NAME)
# GPSIMD cannot access PSUM (BIR verifier) -> evacuations must use ACT
VEVAC = os.environ.get("BASS_VEVAC", "act")     # pool | act
# v matmul layout: "dup" = M=128 [Wv|Wv] so every matmul is 64x128 and
# the PE never switches tiling mode (mode flips cost an array drain);
# "quad" = M=64 with v-hi at tile col 64 sharing one bank
VMODE = os.environ.get("BASS_VMODE", "dup")     # dup | quad | split
# Pool rejects TensorScalarPtr at codegen -> out-STT stays on DVE
POOLOUT = os.environ.get("BASS_POOLOUT", "0") == "1"
# carry the 66-wide H padding through qs/ot/out so every out-STT operand
# is stride-1. Measured: no gain (TensorScalarPtr has no 2x uop; out-STT
# stays ~741ns either way), so default off for the simpler layout.
OUT66 = os.environ.get("BASS_OUT66", "0") == "1"
OW = 66 if OUT66 else H


def _round22(a: np.ndarray) -> np.ndarray:
    """Round fp32 to 11 mantissa bits so the PE's FP22 read-truncation is
    exact (unbiased quantization instead of truncation)."""
    u = np.ascontiguousarray(a, np.float32).view(np.uint32).astype(np.uint64)
    u = ((u + 0x800) & 0xFFFFF000).astype(np.uint32)
    return u.view(np.float32)


def _to_bf16(a: np.ndarray) -> np.ndarray:
    import ml_dtypes
    return np.ascontiguousarray(a, np.float32).astype(ml_dtypes.bfloat16)


def _pack_w(a: np.ndarray) -> np.ndarray:
    return _to_bf16(a) if WDT_NAME == "bf16" else _round22(a)


def _pack_weights(wq, wk, wv, bq, bk, bv, gamma):
    """Pack stationary operands (no bias rows; gamma folded into wv/bv).

    wkq [128, 9, 128]: [Wk | Wq] on both partition halves (k lands on
    psum partitions 0-63 for the DVE accum op, q on 64-127).
    wv2 [128, 9, 64]: gamma*Wv on both halves (M=64).
    bias [128, 3]: col0=bq, col1=bk, col2=gamma*bv, duplicated halves.
    """
    g = float(np.asarray(gamma).reshape(-1)[0])

    def taps(w):  # [O, I, 1, 3, 3] -> [I, 9, O]
        return np.ascontiguousarray(
            w.reshape(C, C, 9).transpose(1, 2, 0), np.float32)

    wq_t, wk_t, wv_t = taps(wq), taps(wk), taps(wv) * g
    # lo chain: [Wk | Wq] (k on psum partitions 0-63); hi chain flipped
    # to [Wq | Wk] so k_{t+1} lands on partitions 64-127 and the whole
    # hi k*v/s pipeline stays on the upper partition half (no s swap)
    wkq = np.zeros((128, NTAP, 128), np.float32)
    wkq[0:64, :, 0:64] = wk_t
    wkq[0:64, :, 64:128] = wq_t
    wkq[64:128, :, 0:64] = wq_t
    wkq[64:128, :, 64:128] = wk_t

    if VMODE == "dup":
        wv2 = np.zeros((128, NTAP, 128), np.float32)
        wv2[0:64, :, 0:64] = wv_t
        wv2[0:64, :, 64:128] = wv_t
        wv2[64:128, :, 0:64] = wv_t
        wv2[64:128, :, 64:128] = wv_t
    else:
        wv2 = np.zeros((128, NTAP, 64), np.float32)
        wv2[0:64] = wv_t
        wv2[64:128] = wv_t

    bias = np.zeros((128, 3), np.float32)
    bias[0:64, 0] = bq
    bias[64:128, 0] = bq
    bias[0:64, 1] = bk
    bias[64:128, 1] = bk
    bias[0:64, 2] = bv * g
    bias[64:128, 2] = bv * g
    return _pack_w(wkq), _pack_w(wv2), bias


def _emit(nc, tc, x_d, wkq_d, wv_d, bias_d, zer_d, out_d, ctx):
    xdt = F32R if XDT_NAME == "f32r" else BF16  # storage dtype of x tiles

    const = ctx.enter_context(tc.tile_pool(name="const", bufs=1))
    state = ctx.enter_context(tc.tile_pool(name="state", bufs=1))
    # 8 PSUM banks total: quad -> kq triple-buffered (6) + v shared (2);
    # dup/split -> kq double (4) + v double (4)
    psum = ctx.enter_context(
        tc.tile_pool(name="psum", bufs=3 if VMODE == "quad" else 2,
                     space=bass.MemorySpace.PSUM))
    psumv = ctx.enter_context(
        tc.tile_pool(name="psumv", bufs=2, space=bass.MemorySpace.PSUM))
    vpool = ctx.enter_context(tc.tile_pool(name="vpool", bufs=2))

    wdt = BF16 if WDT_NAME == "bf16" else F32R
    wkq_t = const.tile([128, NTAP, 128], wdt, tag="wkq")
    wv_t = const.tile([128, NTAP, 128 if VMODE == "dup" else 64], wdt,
                      tag="wv")
    bias_t = const.tile([128, 3], F32, tag="bias")

    odt = F32 if ODT_NAME == "f32r" else BF16

    xp = [state.tile([128, WP, HP], xdt, tag=f"xp{i}", name=f"xp{i}")
          for i in range(3)]
    qs = [state.tile([128, W, OW], odt, tag=f"qs{i}", name=f"qs{i}")
          for i in range(2)]
    ot = [state.tile([128, W, OW], odt, tag=f"ot{i}", name=f"ot{i}")
          for i in range(2)]
    scr = state.tile([128, BN], F32, tag="scr", name="scr")
    sparts = [state.tile([128, NBLK], F32, tag=f"sp{i}", name=f"sp{i}")
              for i in range(2)]
    sfull = [state.tile([128, 1], F32, tag=f"sf{i}", name=f"sf{i}")
             for i in range(2)]

    def load_pair(p):
        t_ = xp[p % 3]
        nc.sync.dma_start(t_[0:64, 1:1 + W, :], x_d[:, 2 * p])
        nc.sync.dma_start(t_[64:128, 1:1 + W, :], x_d[:, 2 * p + 1])

    # wv first on sync (its completion sem gates the first matmul; DMA
    # completion sems lag the transfer by ~4us, so head-of-queue matters)
    nc.sync.dma_start(wv_t[:], wv_d[:])
    # HAM warm-up: burn the free-running 3.4us half-speed window on dummy
    # matmuls while the first loads are in flight. The dummies MUST be the
    # same 64x128 tile shape as every real matmul: a 64x64-shaped attempt
    # flipped the PE tiling mode and regressed the whole body 222->266
    # ns/slot.
    if xdt == BF16 and os.environ.get("BASS_WARM", "1") == "1":
        warm = state.tile([128, BN], xdt, tag="warm", name="warm")
        nc.vector.memset(warm[:, :], 0.0)
        wps = psum.tile([128, BN], F32, tag="kq_lo")
        for _ in range(16):
            nc.tensor.matmul(wps[:, :], warm[0:64, 0:128], warm[0:64, :],
                             start=True, stop=True)
    # zero the W-pad rows once (H-pad columns come zeroed from the host).
    # The BIR verifier rejects compute-engine writes feeding an fp32r
    # matmul, so in f32r mode the zeros come from a host tensor via DMA
    # (on the ACT queue, off the x-load path).
    for t_ in xp:
        if xdt == F32R:
            nc.scalar.dma_start(t_[:, 0, :], zer_d[:, :])
            nc.scalar.dma_start(t_[:, WP - 1, :], zer_d[:, :])
        else:
            nc.vector.memset(t_[:, 0, :], 0.0)
            nc.vector.memset(t_[:, WP - 1, :], 0.0)
    if OUT66:
        # the out-STT reads qs pad columns that the evacs never write
        for qt in qs:
            nc.vector.memset(qt[:, :, 0], 0.0)
            nc.vector.memset(qt[:, :, OW - 1], 0.0)
    # first pair split across all three DMA-capable queues so the first
    # matmul can start after a ~2us quarter-slice load
    HW2 = W // 2
    nc.gpsimd.dma_start(xp[0][0:64, 1:1 + HW2, :], x_d[:, 0, 0:HW2])
    nc.scalar.dma_start(xp[0][0:64, 1 + HW2:1 + W, :], x_d[:, 0, HW2:W])
    nc.sync.dma_start(xp[0][64:128, 1:1 + HW2, :], x_d[:, 1, 0:HW2])
    nc.sync.dma_start(xp[0][64:128, 1 + HW2:1 + W, :], x_d[:, 1, HW2:W])
    nc.gpsimd.dma_start(wkq_t[:], wkq_d[:])
    nc.gpsimd.dma_start(bias_t[:], bias_d[:])
    # load_pair(1) is emitted inside pair 0's block loop: matmuls wait on
    # the issuing queue's DMA counter, so any DMA emitted earlier on the
    # same queue delays the first matmul

    def mm_rhs(xp_, half, tap, j):
        dy, dx = tap // 3, tap % 3
        r0 = j * RB + dy
        return xp_[64 * half:64 * half + 64, r0:r0 + RB, dx:dx + H]

    for p in range(NPAIR):
        pb = p % 2
        xp_, qs_, ot_ = xp[p % 3], qs[pb], ot[pb]

        if p + 2 < NPAIR:
            load_pair(p + 2)

        for j in range(NBLK):
            if p == 0 and j == 2:
                load_pair(1)
            if p == 0 and j == 5:
                load_pair(2)
            if VMODE == "quad":
                v_lo = v_hi = psumv.tile([128, BN], F32, tag="v_lo",
                                         name="v_lo")
                v_lo_out, v_hi_out = v_lo[0:64, :], v_hi[64:128, :]
            elif VMODE == "dup":
                v_lo = psumv.tile([128, BN], F32, tag="v_lo", name="v_lo")
                v_hi = psumv.tile([128, BN], F32, tag="v_hi", name="v_hi")
                v_lo_out, v_hi_out = v_lo[:, :], v_hi[:, :]
            else:
                v_lo = psumv.tile([128, BN], F32, tag="v_lo", name="v_lo")
                v_hi = psumv.tile([128, BN], F32, tag="v_hi", name="v_hi")
                v_lo_out, v_hi_out = v_lo[0:64, :], v_hi[0:64, :]
            kq_lo = psum.tile([128, BN], F32, tag="kq_lo")
            kq_hi = psum.tile([128, BN], F32, tag="kq_hi")

            for tap in range(NTAP):
                st, sp = tap == 0, tap == NTAP - 1
                nc.tensor.matmul(
                    v_lo_out, wv_t[0:64, tap, :],
                    mm_rhs(xp_, 0, tap, j), start=st, stop=sp)
                nc.tensor.matmul(
                    v_hi_out, wv_t[64:128, tap, :],
                    mm_rhs(xp_, 1, tap, j), start=st, stop=sp)
            for tap in range(NTAP):
                st, sp = tap == 0, tap == NTAP - 1
                nc.tensor.matmul(
                    kq_lo[:, :], wkq_t[0:64, tap, :],
                    mm_rhs(xp_, 0, tap, j), start=st, stop=sp)
                nc.tensor.matmul(
                    kq_hi[:, :], wkq_t[64:128, tap, :],
                    mm_rhs(xp_, 1, tap, j), start=st, stop=sp)

            # v + bv -> SBUF (ACT), q + bq -> SBUF (ACT; lo crosses
            # partitions 64-127 -> 0-63 to line up with x_t)
            vsb = vpool.tile([128, BN], F32, tag="vsb", name="vsb")
            if VMODE == "quad":
                nc.scalar.activation(
                    vsb[:, :], v_lo[:, :], ACTF.Identity,
                    bias=bias_t[:, 2:3])
            elif VMODE == "dup":
                # dup layout puts v_{t+1} on partitions 64-127 of its own
                # bank too, so both evacs are partition-aligned
                nc.scalar.activation(
                    vsb[0:64, :], v_lo[0:64, :], ACTF.Identity,
                    bias=bias_t[0:64, 2:3])
                nc.scalar.activation(
                    vsb[64:128, :], v_hi[64:128, :], ACTF.Identity,
                    bias=bias_t[64:128, 2:3])
            else:
                nc.scalar.activation(
                    vsb[0:64, :], v_lo[0:64, :], ACTF.Identity,
                    bias=bias_t[0:64, 2:3])
                nc.scalar.activation(
                    vsb[64:128, :], v_hi[0:64, :], ACTF.Identity,
                    bias=bias_t[64:128, 2:3])
            qoff = 1 if OUT66 else 0
            nc.scalar.activation(
                qs_[0:64, j * RB:(j + 1) * RB, qoff:qoff + H],
                kq_lo[64:128, :], ACTF.Identity, bias=bias_t[64:128, 0:1])
            nc.scalar.activation(
                qs_[64:128, j * RB:(j + 1) * RB, qoff:qoff + H],
                kq_hi[0:64, :], ACTF.Identity, bias=bias_t[0:64, 0:1])

            # (k+bk)*v with pixel-sum accumulation; the lo chain lives on
            # partitions 0-63, the hi chain on 64-127 throughout
            nc.vector.scalar_tensor_tensor(
                out=scr[0:64, :], in0=kq_lo[0:64, :],
                scalar=bias_t[0:64, 1:2], in1=vsb[0:64, :],
                op0=ALU.add, op1=ALU.mult,
                accum_out=sparts[pb][0:64, j:j + 1])
            nc.vector.scalar_tensor_tensor(
                out=scr[64:128, :], in0=kq_hi[64:128, :],
                scalar=bias_t[64:128, 1:2], in1=vsb[64:128, :],
                op0=ALU.add, op1=ALU.mult,
                accum_out=sparts[pb][64:128, j:j + 1])

        nc.vector.tensor_reduce(
            sfull[pb][:, 0:1], sparts[pb][:, :],
            axis=mybir.AxisListType.X, op=ALU.add)

        for j in range(NBLK):
            if OUT66:
                # every operand stride-1 over the 66-wide padded rows
                in1 = xp_[:, 1 + j * RB:1 + (j + 1) * RB, :]
                out_ap = ot_[:, j * RB:(j + 1) * RB, :]
                in0_ap = qs_[:, j * RB:(j + 1) * RB, :]
            else:
                in1 = xp_[:, 1 + j * RB:1 + (j + 1) * RB, 1:1 + H]
                out_ap = ot_[:, j * RB:(j + 1) * RB, 0:H]
                in0_ap = qs_[:, j * RB:(j + 1) * RB, 0:H]
            if xdt == F32R:
                in1 = in1.bitcast(F32)
            nc.vector.scalar_tensor_tensor(
                out=out_ap,
                in0=in0_ap,
                scalar=sfull[pb][:, 0:1],
                in1=in1,
                op0=ALU.mult, op1=ALU.add)

        # out-DMAs on sync (x loads leave it mostly idle). Putting the hi
        # DMA on the scalar queue was tried: it bubbles the ACT evac
        # stream and cost ~12us of scattered PE stalls, with no tail gain.
        nc.sync.dma_start(out_d[:, 2 * p], ot_[0:64, :])
        nc.sync.dma_start(out_d[:, 2 * p + 1], ot_[64:128, :])


_CACHE = {}


def _build():
    key = (XDT_NAME, WDT_NAME, VEVAC, VMODE)
    if key in _CACHE:
        return _CACHE[key]
    nc = bacc.Bacc("TRN2", target_bir_lowering=False, debug=False,
                   enable_asserts=False, num_devices=8)
    xdt = F32R if XDT_NAME == "f32r" else BF16
    wdt = BF16 if WDT_NAME == "bf16" else F32R
    x_d = nc.dram_tensor("xpad", (C, T, W, HP), xdt,
                         kind="ExternalInput").ap()
    wkq_d = nc.dram_tensor("wkq", (128, NTAP, 128), wdt,
                           kind="ExternalInput").ap()
    wv_d = nc.dram_tensor("wv2", (128, NTAP, 128 if VMODE == "dup" else 64),
                          wdt, kind="ExternalInput").ap()
    bias_d = nc.dram_tensor("biases", (128, 3), F32,
                            kind="ExternalInput").ap()
    zer_d = nc.dram_tensor("zer", (128, HP), xdt,
                           kind="ExternalInput").ap()
    odt = F32 if ODT_NAME == "f32r" else BF16
    out_d = nc.dram_tensor("out", (C, T, W, OW), odt,
                           kind="ExternalOutput").ap()
    from contextlib import ExitStack
    with tile.TileContext(nc) as tc, ExitStack() as ctx:
        _emit(nc, tc, x_d, wkq_d, wv_d, bias_d, zer_d, out_d, ctx)
    nc.compile()
    _CACHE[key] = nc
    return nc


def run_spmd(x, wq, wk, wv, bq, bk, bv, gamma, trace=False, **kw):
    nc = _build()
    wkq, wv2, biases = _pack_weights(
        np.asarray(wq, np.float32), np.asarray(wk, np.float32),
        np.asarray(wv, np.float32), np.asarray(bq, np.float32),
        np.asarray(bk, np.float32), np.asarray(bv, np.float32),
        np.asarray(gamma, np.float32))
    x = np.asarray(x, np.float32)
    xpad = np.zeros((B, C, T, W, HP), np.float32)
    xpad[..., 1:1 + H] = x
    zer = np.zeros((128, HP), np.float32)
    if XDT_NAME == "bf16":
        xpad = _to_bf16(xpad)
        zer = _to_bf16(zer)
    in_maps = [
        {"xpad": np.ascontiguousarray(xpad[b]), "wkq": wkq, "wv2": wv2,
         "biases": biases, "zer": zer}
        for b in range(B)
    ]
    res = bass_utils.run_bass_kernel_spmd(
        nc, in_maps, core_ids=list(range(B)), trace=trace, **kw)
    out = np.stack(
        [np.asarray(res.results[b]["out"], np.float32) for b in range(B)],
        axis=0)
    if OUT66:
        out = np.ascontiguousarray(out[..., 1:1 + H])
    return out, res


def kernel(x, wq, wk, wv, bq, bk, bv, gamma):
    out, _ = run_spmd(x, wq, wk, wv, bq, bk, bv, gamma)
    return out


# revision 58
# speedup vs baseline: 1.0316x; 1.0099x over previous
"""Trainium2 Bass kernel for conv-qkv rank-1 attention.

out = gamma * (q+bq) * sum((k+bk)*(v+bv)) + x, where q,k,v are
per-time-slice 3x3 convs (C=64 -> C=64) of x [B=8, C=64, T=16, W=64, H=64].

Sharding: data-parallel over B across 8 cores (1 example/core), conv
weights replicated. No cross-core communication.

Design (~285us vs ~492us v1 baseline; PE-bound at the hardware floor):
- Slice pair per pass: slice t on SBUF partitions 0-63, t+1 on 64-127;
  the two 64-row PE tile chains stream concurrently (input port limit:
  two K=64 streams saturate the 128-partition rhs port).
- 18 pair-slots per (pair, 512-px block): 9 taps x ([k|q] M=128 +
  [v|v] M=128). No bias taps: bq/bv fold into the ACT evacuation bias,
  bk into the DVE STT op0-add scalar, gamma into wv/bv host-side.
- All matmuls are 64x128 tiles (v weights column-duplicated): mixing
  64x64 and 64x128 shapes flips the PE tiling mode, whose drain broke
  fill/drain overlap (222 -> 218 ns/slot once uniform; theoretical
  floor is 512/2.4GHz + ~3 NX cycles = 216).
- bf16 end-to-end (PE is 1 cyc/row for both f32r and bf16, but bf16
  LDWEIGHTS is half the load and draws less power -> less HAM
  throttling; absmax rel err 5.3e-3 vs 2e-2 gate).
- Hi chain stationary flipped to [Wq|Wk] so the k*v STT, s accumulator
  and q all live on partitions 64-127: no cross-partition s swap.
- Host pads H to 66 so each x slice loads with ONE contiguous
  descriptor per partition (was 64 x 256B strided descriptors -> 57us
  serial startup and 205us of DMA activity).
- Startup: matmuls wait on per-queue DMA completion counters (+~4.5us
  sem latency), so wv goes first on sync, the first pair is split
  across all 3 DMA queues, and later loads are emitted behind the
  first blocks' matmuls. First matmul at ~13us.
- Epilogue per pair: one [128,8] reduce -> s, merged [128,512]
  q*s+x STTs on DVE, out-DMAs on sync.
"""

import os

import numpy as np

import concourse.bacc as bacc
import concourse.bass as bass
import concourse.mybir as mybir
import concourse.tile as tile
from concourse import bass_utils

F32 = mybir.dt.float32
F32R = mybir.dt.float32r
BF16 = mybir.dt.bfloat16
ALU = mybir.AluOpType
ACTF = mybir.ActivationFunctionType

B, C, T, W, H = 8, 64, 16, 64, 64
HP = H + 2                     # host-padded H
WP = W + 2                     # SBUF-padded W rows
NPAIR = T // 2                 # slice pairs per core
RB = 8                         # W-rows per pixel block
NBLK = W // RB                 # pixel blocks per slice
BN = RB * H                    # moving free dim per matmul (512)
NTAP = 9                       # conv taps (no bias tap)

XDT_NAME = os.environ.get("BASS_XDT", "bf16")   # moving/x dtype
# walrus rejects mixed 32/16-bit matmul inputs: stationary follows moving
WDT_NAME = os.environ.get("BASS_WDT", "bf16" if XDT_NAME == "bf16" else "f32r")
# out/qs storage dtype follows x by default
ODT_NAME = os.environ.get("BASS_ODT", XDT_NAME)
# GPSIMD cannot access PSUM (BIR verifier) -> evacuations must use ACT
VEVAC = os.environ.get("BASS_VEVAC", "act")     # pool | act
# v matmul layout: "dup" = M=128 [Wv|Wv] so every matmul is 64x128 and
# the PE never switches tiling mode (mode flips cost an array drain);
# "quad" = M=64 with v-hi at tile col 64 sharing one bank
VMODE = os.environ.get("BASS_VMODE", "dup")     # dup | quad | split
# Pool rejects TensorScalarPtr at codegen -> out-STT stays on DVE
POOLOUT = os.environ.get("BASS_POOLOUT", "0") == "1"
# carry the 66-wide H padding through qs/ot/out so every out-STT operand
# is stride-1. Measured: no gain (TensorScalarPtr has no 2x uop; out-STT
# stays ~741ns either way), so default off for the simpler layout.
OUT66 = os.environ.get("BASS_OUT66", "0") == "1"
OW = 66 if OUT66 else H


def _round22(a: np.ndarray) -> np.ndarray:
    """Round fp32 to 11 mantissa bits so the PE's FP22 read-truncation is
    exact (unbiased quantization instead of truncation)."""
    u = np.ascontiguousarray(a, np.float32).view(np.uint32).astype(np.uint64)
    u = ((u + 0x800) & 0xFFFFF000).astype(np.uint32)
    return u.view(np.float32)


def _to_bf16(a: np.ndarray) -> np.ndarray:
    import ml_dtypes
    return np.ascontiguousarray(a, np.float32).astype(ml_dtypes.bfloat16)


def _pack_w(a: np.ndarray) -> np.ndarray:
    return _to_bf16(a) if WDT_NAME == "bf16" else _round22(a)


def _pack_weights(wq, wk, wv, bq, bk, bv, gamma):
    """Pack stationary operands (no bias rows; gamma folded into wv/bv).

    wkq [128, 9, 128]: [Wk | Wq] on both partition halves (k lands on
    psum partitions 0-63 for the DVE accum op, q on 64-127).
    wv2 [128, 9, 64]: gamma*Wv on both halves (M=64).
    bias [128, 3]: col0=bq, col1=bk, col2=gamma*bv, duplicated halves.
    """
    g = float(np.asarray(gamma).reshape(-1)[0])

    def taps(w):  # [O, I, 1, 3, 3] -> [I, 9, O]
        return np.ascontiguousarray(
            w.reshape(C, C, 9).transpose(1, 2, 0), np.float32)

    wq_t, wk_t, wv_t = taps(wq), taps(wk), taps(wv) * g
    # lo chain: [Wk | Wq] (k on psum partitions 0-63); hi chain flipped
    # to [Wq | Wk] so k_{t+1} lands on partitions 64-127 and the whole
    # hi k*v/s pipeline stays on the upper partition half (no s swap)
    wkq = np.zeros((128, NTAP, 128), np.float32)
    wkq[0:64, :, 0:64] = wk_t
    wkq[0:64, :, 64:128] = wq_t
    wkq[64:128, :, 0:64] = wq_t
    wkq[64:128, :, 64:128] = wk_t

    if VMODE == "dup":
        wv2 = np.zeros((128, NTAP, 128), np.float32)
        wv2[0:64, :, 0:64] = wv_t
        wv2[0:64, :, 64:128] = wv_t
        wv2[64:128, :, 0:64] = wv_t
        wv2[64:128, :, 64:128] = wv_t
    else:
        wv2 = np.zeros((128, NTAP, 64), np.float32)
        wv2[0:64] = wv_t
        wv2[64:128] = wv_t

    bias = np.zeros((128, 3), np.float32)
    bias[0:64, 0] = bq
    bias[64:128, 0] = bq
    bias[0:64, 1] = bk
    bias[64:128, 1] = bk
    bias[0:64, 2] = bv * g
    bias[64:128, 2] = bv * g
    return _pack_w(wkq), _pack_w(wv2), bias


def _emit(nc, tc, x_d, wkq_d, wv_d, bias_d, zer_d, out_d, ctx):
    xdt = F32R if XDT_NAME == "f32r" else BF16  # storage dtype of x tiles

    const = ctx.enter_context(tc.tile_pool(name="const", bufs=1))
    state = ctx.enter_context(tc.tile_pool(name="state", bufs=1))
    # 8 PSUM banks total: quad -> kq triple-buffered (6) + v shared (2);
    # dup/split -> kq double (4) + v double (4)
    psum = ctx.enter_context(
        tc.tile_pool(name="psum", bufs=3 if VMODE == "quad" else 2,
                     space=bass.MemorySpace.PSUM))
    psumv = ctx.enter_context(
        tc.tile_pool(name="psumv", bufs=2, space=bass.MemorySpace.PSUM))
    vpool = ctx.enter_context(tc.tile_pool(name="vpool", bufs=2))

    wdt = BF16 if WDT_NAME == "bf16" else F32R
    wkq_t = const.tile([128, NTAP, 128], wdt, tag="wkq")
    wv_t = const.tile([128, NTAP, 128 if VMODE == "dup" else 64], wdt,
                      tag="wv")
    bias_t = const.tile([128, 3], F32, tag="bias")

    odt = F32 if ODT_NAME == "f32r" else BF16

    xp = [state.tile([128, WP, HP], xdt, tag=f"xp{i}", name=f"xp{i}")
          for i in range(3)]
    qs = [state.tile([128, W, OW], odt, tag=f"qs{i}", name=f"qs{i}")
          for i in range(2)]
    ot = [state.tile([128, W, OW], odt, tag=f"ot{i}", name=f"ot{i}")
          for i in range(2)]
    scr = state.tile([128, BN], F32, tag="scr", name="scr")
    sparts = [state.tile([128, NBLK], F32, tag=f"sp{i}", name=f"sp{i}")
              for i in range(2)]
    sfull = [state.tile([128, 1], F32, tag=f"sf{i}", name=f"sf{i}")
             for i in range(2)]

    def load_pair(p):
        t_ = xp[p % 3]
        nc.sync.dma_start(t_[0:64, 1:1 + W, :], x_d[:, 2 * p])
        nc.sync.dma_start(t_[64:128, 1:1 + W, :], x_d[:, 2 * p + 1])

    # wv first on sync (its completion sem gates the first matmul; DMA
    # completion sems lag the transfer by ~4us, so head-of-queue matters)
    nc.sync.dma_start(wv_t[:], wv_d[:])
    # HAM warm-up: burn the free-running 3.4us half-speed window on dummy
    # matmuls while the first loads are in flight. The dummies MUST be the
    # same 64x128 tile shape as every real matmul: a 64x64-shaped attempt
    # flipped the PE tiling mode and regressed the whole body 222->266
    # ns/slot.
    # Net effect measured NEGATIVE (+3us): the early start (13.7->7.9us)
    # is outweighed by scattered PE waits from the extra kq-pool
    # allocation shifting psum bank phase. Default off.
    if xdt == BF16 and os.environ.get("BASS_WARM", "0") == "1":
        warm = state.tile([128, BN], xdt, tag="warm", name="warm")
        nc.vector.memset(warm[:, :], 0.0)
        wps = psum.tile([128, BN], F32, tag="kq_lo")
        for _ in range(16):
            nc.tensor.matmul(wps[:, :], warm[0:64, 0:128], warm[0:64, :],
                             start=True, stop=True)
    # zero the W-pad rows once (H-pad columns come zeroed from the host).
    # The BIR verifier rejects compute-engine writes feeding an fp32r
    # matmul, so in f32r mode the zeros come from a host tensor via DMA
    # (on the ACT queue, off the x-load path).
    for t_ in xp:
        if xdt == F32R:
            nc.scalar.dma_start(t_[:, 0, :], zer_d[:, :])
            nc.scalar.dma_start(t_[:, WP - 1, :], zer_d[:, :])
        else:
            nc.vector.memset(t_[:, 0, :], 0.0)
            nc.vector.memset(t_[:, WP - 1, :], 0.0)
    if OUT66:
        # the out-STT reads qs pad columns that the evacs never write
        for qt in qs:
            nc.vector.memset(qt[:, :, 0], 0.0)
            nc.vector.memset(qt[:, :, OW - 1], 0.0)
    # first pair split across all three DMA-capable queues so the first
    # matmul can start after a ~2us quarter-slice load
    HW2 = W // 2
    nc.gpsimd.dma_start(xp[0][0:64, 1:1 + HW2, :], x_d[:, 0, 0:HW2])
    nc.scalar.dma_start(xp[0][0:64, 1 + HW2:1 + W, :], x_d[:, 0, HW2:W])
    nc.sync.dma_start(xp[0][64:128, 1:1 + HW2, :], x_d[:, 1, 0:HW2])
    nc.sync.dma_start(xp[0][64:128, 1 + HW2:1 + W, :], x_d[:, 1, HW2:W])
    nc.gpsimd.dma_start(wkq_t[:], wkq_d[:])
    nc.gpsimd.dma_start(bias_t[:], bias_d[:])
    # load_pair(1) is emitted inside pair 0's block loop: matmuls wait on
    # the issuing queue's DMA counter, so any DMA emitted earlier on the
    # same queue delays the first matmul

    def mm_rhs(xp_, half, tap, j):
        dy, dx = tap // 3, tap % 3
        r0 = j * RB + dy
        return xp_[64 * half:64 * half + 64, r0:r0 + RB, dx:dx + H]

    for p in range(NPAIR):
        pb = p % 2
        xp_, qs_, ot_ = xp[p % 3], qs[pb], ot[pb]

        if p + 2 < NPAIR:
            load_pair(p + 2)

        for j in range(NBLK):
            if p == 0 and j == 2:
                load_pair(1)
            if p == 0 and j == 5:
                load_pair(2)
            if VMODE == "quad":
                v_lo = v_hi = psumv.tile([128, BN], F32, tag="v_lo",
                                         name="v_lo")
                v_lo_out, v_hi_out = v_lo[0:64, :], v_hi[64:128, :]
            elif VMODE == "dup":
                v_lo = psumv.tile([128, BN], F32, tag="v_lo", name="v_lo")
                v_hi = psumv.tile([128, BN], F32, tag="v_hi", name="v_hi")
                v_lo_out, v_hi_out = v_lo[:, :], v_hi[:, :]
            else:
                v_lo = psumv.tile([128, BN], F32, tag="v_lo", name="v_lo")
                v_hi = psumv.tile([128, BN], F32, tag="v_hi", name="v_hi")
                v_lo_out, v_hi_out = v_lo[0:64, :], v_hi[0:64, :]
            kq_lo = psum.tile([128, BN], F32, tag="kq_lo")
            kq_hi = psum.tile([128, BN], F32, tag="kq_hi")

            for tap in range(NTAP):
                st, sp = tap == 0, tap == NTAP - 1
                nc.tensor.matmul(
                    v_lo_out, wv_t[0:64, tap, :],
                    mm_rhs(xp_, 0, tap, j), start=st, stop=sp)
                nc.tensor.matmul(
                    v_hi_out, wv_t[64:128, tap, :],
                    mm_rhs(xp_, 1, tap, j), start=st, stop=sp)
            for tap in range(NTAP):
                st, sp = tap == 0, tap == NTAP - 1
                nc.tensor.matmul(
                    kq_lo[:, :], wkq_t[0:64, tap, :],
                    mm_rhs(xp_, 0, tap, j), start=st, stop=sp)
                nc.tensor.matmul(
                    kq_hi[:, :], wkq_t[64:128, tap, :],
                    mm_rhs(xp_, 1, tap, j), start=st, stop=sp)

            # v + bv -> SBUF (ACT), q + bq -> SBUF (ACT; lo crosses
            # partitions 64-127 -> 0-63 to line up with x_t)
            vsb = vpool.tile([128, BN], F32, tag="vsb", name="vsb")
            if VMODE == "quad":
                nc.scalar.activation(
                    vsb[:, :], v_lo[:, :], ACTF.Identity,
                    bias=bias_t[:, 2:3])
            elif VMODE == "dup":
                # dup layout puts v_{t+1} on partitions 64-127 of its own
                # bank too, so both evacs are partition-aligned
                nc.scalar.activation(
                    vsb[0:64, :], v_lo[0:64, :], ACTF.Identity,
                    bias=bias_t[0:64, 2:3])
                nc.scalar.activation(
                    vsb[64:128, :], v_hi[64:128, :], ACTF.Identity,
                    bias=bias_t[64:128, 2:3])
            else:
                nc.scalar.activation(
                    vsb[0:64, :], v_lo[0:64, :], ACTF.Identity,
                    bias=bias_t[0:64, 2:3])
                nc.scalar.activation(
                    vsb[64:128, :], v_hi[0:64, :], ACTF.Identity,
                    bias=bias_t[64:128, 2:3])
            qoff = 1 if OUT66 else 0
            nc.scalar.activation(
                qs_[0:64, j * RB:(j + 1) * RB, qoff:qoff + H],
                kq_lo[64:128, :], ACTF.Identity, bias=bias_t[64:128, 0:1])
            nc.scalar.activation(
                qs_[64:128, j * RB:(j + 1) * RB, qoff:qoff + H],
                kq_hi[0:64, :], ACTF.Identity, bias=bias_t[0:64, 0:1])

            # (k+bk)*v with pixel-sum accumulation; the lo chain lives on
            # partitions 0-63, the hi chain on 64-127 throughout
            nc.vector.scalar_tensor_tensor(
                out=scr[0:64, :], in0=kq_lo[0:64, :],
                scalar=bias_t[0:64, 1:2], in1=vsb[0:64, :],
                op0=ALU.add, op1=ALU.mult,
                accum_out=sparts[pb][0:64, j:j + 1])
            nc.vector.scalar_tensor_tensor(
                out=scr[64:128, :], in0=kq_hi[64:128, :],
                scalar=bias_t[64:128, 1:2], in1=vsb[64:128, :],
                op0=ALU.add, op1=ALU.mult,
                accum_out=sparts[pb][64:128, j:j + 1])

        nc.vector.tensor_reduce(
            sfull[pb][:, 0:1], sparts[pb][:, :],
            axis=mybir.AxisListType.X, op=ALU.add)

        for j in range(NBLK):
            if OUT66:
                # every operand stride-1 over the 66-wide padded rows
                in1 = xp_[:, 1 + j * RB:1 + (j + 1) * RB, :]
                out_ap = ot_[:, j * RB:(j + 1) * RB, :]
                in0_ap = qs_[:, j * RB:(j + 1) * RB, :]
            else:
                in1 = xp_[:, 1 + j * RB:1 + (j + 1) * RB, 1:1 + H]
                out_ap = ot_[:, j * RB:(j + 1) * RB, 0:H]
                in0_ap = qs_[:, j * RB:(j + 1) * RB, 0:H]
            if xdt == F32R:
                in1 = in1.bitcast(F32)
            nc.vector.scalar_tensor_tensor(
                out=out_ap,
                in0=in0_ap,
                scalar=sfull[pb][:, 0:1],
                in1=in1,
                op0=ALU.mult, op1=ALU.add)

        # out-DMAs on sync (x loads leave it mostly idle). Putting the hi
        # DMA on the scalar queue was tried: it bubbles the ACT evac
        # stream and cost ~12us of scattered PE stalls, with no tail gain.
        nc.sync.dma_start(out_d[:, 2 * p], ot_[0:64, :])
        nc.sync.dma_start(out_d[:, 2 * p + 1], ot_[64:128, :])


_CACHE = {}


def _build():
    key = (XDT_NAME, WDT_NAME, VEVAC, VMODE)
    if key in _CACHE:
        return _CACHE[key]
    nc = bacc.Bacc("TRN2", target_bir_lowering=False, debug=False,
                   enable_asserts=False, num_devices=8)
    xdt = F32R if XDT_NAME == "f32r" else BF16
    wdt = BF16 if WDT_NAME == "bf16" else F32R
    x_d = nc.dram_tensor("xpad", (C, T, W, HP), xdt,
                         kind="ExternalInput").ap()
    wkq_d = nc.dram_tensor("wkq", (128, NTAP, 128), wdt,
                           kind="ExternalInput").ap()
    wv_d = nc.dram_tensor("wv2", (128, NTAP, 128 if VMODE == "dup" else 64),
                          wdt, kind="ExternalInput").ap()
    bias_d = nc.dram_tensor("biases", (128, 3), F32,
                            kind="ExternalInput").ap()
    zer_d = nc.dram_tensor("zer", (128, HP), xdt,
                           kind="ExternalInput").ap()
    odt = F32 if ODT_NAME == "f32r" else BF16
    out_d = nc.dram_tensor("out", (C, T, W, OW), odt,
                           kind="ExternalOutput").ap()
    from contextlib import ExitStack
    with tile.TileContext(nc) as tc, ExitStack() as ctx:
        _emit(nc, tc, x_d, wkq_d, wv_d, bias_d, zer_d, out_d, ctx)
    nc.compile()
    _CACHE[key] = nc
    return nc


def run_spmd(x, wq, wk, wv, bq, bk, bv, gamma, trace=False, **kw):
    nc = _build()
    wkq, wv2, biases = _pack_weights(
        np.asarray(wq, np.float32), np.asarray(wk, np.float32),
        np.asarray(wv, np.float32), np.asarray(bq, np.float32),
        np.asarray(bk, np.float32), np.asarray(bv, np.float32),
        np.asarray(gamma, np.float32))
    x = np.asarray(x, np.float32)
    xpad = np.zeros((B, C, T, W, HP), np.float32)
    xpad[..., 1:1 + H] = x
    zer = np.zeros((128, HP), np.float32)
    if XDT_NAME == "bf16":
        xpad = _to_bf16(xpad)
        zer = _to_bf16(zer)
    in_maps = [
        {"xpad": np.ascontiguousarray(xpad[b]), "wkq": wkq, "wv2": wv2,
         "biases": biases, "zer": zer}
        for b in range(B)
    ]
    res = bass_utils.run_bass_kernel_spmd(
        nc, in_maps, core_ids=list(range(B)), trace=trace, **kw)
    out = np.stack(
        [np.asarray(res.results[b]["out"], np.float32) for b in range(B)],
        axis=0)
    if OUT66:
        out = np.ascontiguousarray(out[..., 1:1 + H])
    return out, res


def kernel(x, wq, wk, wv, bq, bk, bv, gamma):
    out, _ = run_spmd(x, wq, wk, wv, bq, bk, bv, gamma)
    return out
